# revision 1
# baseline (speedup 1.0000x reference)
"""Trainium2 Bass kernel for the ReActNet-style binary conv building block.

Strategy: pure data-parallel across 8 NeuronCores (8 samples each).
All heavy math is done as bf16 matmuls over binarized {0,1} activations
(b = (x>=0), padding encoded as 0.5 so that 2*W@b - rowsum(W) == conv of
sign(x) with zero padding). BN/RPReLU/shortcut-BN are folded on the host
into per-channel scale/bias vectors applied by the Scalar engine
(Relu activations) and fused DVE ops. quant4 uses the bf16 cast-rounding
trick: bf16(7.5*x + 199.5) rounds to the integer grid exactly.
"""

import sys

sys.path.insert(0, "/opt/trn_rl_repo")

import numpy as np
import ml_dtypes

B_PER_CORE = 8
N_CORES = 8
CIN = 256
COUT = 512
H = 28
W = 28
HO = 14
WO = 14
PIX = HO * WO  # 196
GROUPS = 4  # sample pairs per core
NG = 2  # samples per group
NCOL = NG * PIX  # 392 matmul free size

# padded image layout (rows 0..29, cols 0..31); interior at [1:29, 2:30]
PH, PW = 30, 32

_PROGRAM_CACHE = {}


def _build_program():
    """Build + compile the single-core Bass/Tile program (same on all cores)."""
    if "nc" in _PROGRAM_CACHE:
        return _PROGRAM_CACHE["nc"]

    import concourse.bacc as bacc
    import concourse.tile as tile
    from concourse import mybir

    f32 = mybir.dt.float32
    bf16 = mybir.dt.bfloat16
    Alu = mybir.AluOpType
    Act = mybir.ActivationFunctionType

    nc = bacc.Bacc(
        "TRN2",
        target_bir_lowering=False,
        debug=False,
        enable_asserts=False,
        num_devices=N_CORES,
    )

    xs_d = nc.dram_tensor("xs", [B_PER_CORE, 2, 128, H * W], f32, kind="ExternalInput")
    w3_d = nc.dram_tensor("w3s", [128, 2 * 18 * 128], bf16, kind="ExternalInput")
    w1_d = nc.dram_tensor("w1s", [128, 2 * 4 * 128], bf16, kind="ExternalInput")
    cv_d = nc.dram_tensor("cv", [128, 38], f32, kind="ExternalInput")
    dg_d = nc.dram_tensor("dg", [128, 128], bf16, kind="ExternalInput")
    out_d = nc.dram_tensor(
        "out", [B_PER_CORE, 4, 128, PIX], f32, kind="ExternalOutput"
    )

    with tile.TileContext(nc) as tc:
        with (
            tc.tile_pool(name="consts", bufs=1) as cpool,
            tc.tile_pool(name="xin", bufs=3) as xpool,
            tc.tile_pool(name="rq", bufs=3) as rpool,
            tc.tile_pool(name="bpad", bufs=3) as bpool,
            tc.tile_pool(name="rcq", bufs=3) as rcpool,
            tc.tile_pool(name="gact", bufs=4) as gpool,
            tc.tile_pool(name="ymid", bufs=3) as ypool,
            tc.tile_pool(name="quant2", bufs=3) as qpool,
            tc.tile_pool(name="zact", bufs=3) as zpool,
            tc.tile_pool(name="outs", bufs=3) as opool,
            tc.tile_pool(name="pc1", bufs=3, space="PSUM") as pc1,
            tc.tile_pool(name="pc2", bufs=2, space="PSUM") as pc2,
            tc.tile_pool(name="pq", bufs=3, space="PSUM") as pq,
        ):
            W3S = cpool.tile([128, 2 * 18 * 128], bf16)
            W1S = cpool.tile([128, 2 * 4 * 128], bf16)
            CV = cpool.tile([128, 38], f32)
            DG = cpool.tile([128, 128], bf16)
            nc.sync.dma_start(W3S[:], w3_d[:])
            nc.sync.dma_start(W1S[:], w1_d[:])
            nc.sync.dma_start(CV[:], cv_d[:])
            nc.sync.dma_start(DG[:], dg_d[:])

            def cvec(col):
                return CV[:, col : col + 1]

            for g4 in range(2):
                y4 = [
                    ypool.tile([128, 2, NCOL], f32, tag=f"y4_{j}",
                               name=f"y4_{g4}_{j}")
                    for j in range(2)
                ]
                s24 = [
                    qpool.tile([128, 2, NCOL], bf16, tag=f"s24_{j}",
                               name=f"s24_{g4}_{j}")
                    for j in range(2)
                ]
                rc24 = [
                    qpool.tile([128, 2, NCOL], bf16, tag=f"rc24_{j}",
                               name=f"rc24_{g4}_{j}")
                    for j in range(2)
                ]
                zA4 = [
                    zpool.tile([128, 2, NCOL], bf16, tag=f"zA4_{jj}",
                               name=f"zA4_{g4}_{jj}")
                    for jj in range(4)
                ]
                zB4 = [
                    zpool.tile([128, 2, NCOL], bf16, tag=f"zB4_{jj}",
                               name=f"zB4_{g4}_{jj}")
                    for jj in range(4)
                ]
                for h in range(2):
                    g = 2 * g4 + h
                    BP = bpool.tile([128, 2, NG, PH, PW], bf16, tag="bpad")
                    # zero-pad ring cells the conv taps can read
                    nc.gpsimd.memset(BP[:, :, :, 0, :], 0.0)
                    nc.gpsimd.memset(BP[:, :, :, 1:29, 1], 0.0)
                    Q2p = [
                        pq.tile([128, 512], f32, tag="pq", name=f"q2p_{g}_{jq}")
                        for jq in range(2)
                    ]
                    RCg = rcpool.tile([128, 2, NG, H * W], bf16, tag="rc")
                    for si in range(NG):
                        s = NG * g + si
                        X = xpool.tile([128, 2, H * W], f32, tag="x")
                        nc.sync.dma_start(
                            X[:], xs_d[s].rearrange("c p hw -> p c hw")
                        )
                        # R = bf16(7.5*x + 199.5): rounds to int grid (r+192)
                        R = rpool.tile([128, 2, H * W], bf16, tag="r")
                        nc.vector.tensor_scalar(
                            R[:], X[:], 7.5, 199.5, Alu.mult, Alu.add
                        )
                        # sign(x) in {-1,+1} into zero-padded tile (ACT)
                        Xv = X[:].rearrange("p c (h w) -> p c h w", h=H, w=W)
                        nc.scalar.sign(BP[:, :, si, 1:29, 2:30], Xv)
                        # clipped quant values (r+192 in [192,207])
                        nc.vector.tensor_scalar(
                            RCg[:, :, si, :], R[:], 207.0, 192.0,
                            Alu.min, Alu.max,
                        )

                    # 2x2 sum-pool into PSUM via identity-diag matmuls
                    RCgv = RCg[:].rearrange(
                        "p c s (y a x b) -> p c s y a x b", y=HO, a=2, x=WO, b=2
                    )
                    for jt in range(8):
                        j, ph, pw = jt >> 2, (jt >> 1) & 1, jt & 1
                        nc.tensor.matmul(
                            Q2p[j][:, :NCOL].rearrange(
                                "p (s y x) -> p s y x", s=NG, y=HO
                            ),
                            DG[:],
                            RCgv[:, j, :, :, ph, :, pw],
                            start=((jt & 3) == 0),
                            stop=((jt & 3) == 3),
                        )

                    # conv1: 18 accumulated matmuls per output-channel half
                    for j in range(2):
                        ps1 = pc1.tile([128, 512], f32, tag="ps1")
                        out_mm = ps1[:, :NCOL].rearrange(
                            "p (s y x) -> p s y x", s=NG, y=HO, x=WO
                        )
                        for n_mm in range(18):
                            c, kh, kw = n_mm // 9, (n_mm // 3) % 3, n_mm % 3
                            idx = ((j * 2 + c) * 3 + kh) * 3 + kw
                            rhs = (
                                BP[:, c, :, kh : kh + 28, kw + 1 : kw + 29]
                                .rearrange(
                                    "p s (y a) (x b) -> p s y a x b", a=2, b=2
                                )[:, :, :, 0, :, 0]
                            )
                            nc.tensor.matmul(
                                out_mm,
                                W3S[:, idx * 128 : (idx + 1) * 128],
                                rhs,
                                start=(n_mm == 0),
                                stop=(n_mm == 17),
                            )
                        # gA = relu(t*sinv1), gB = relu(-t*beta1*sinv1)
                        gA = gpool.tile([128, NCOL], f32, tag="gA")
                        nc.scalar.activation(
                            gA[:], ps1[:, :NCOL], Act.Relu,
                            bias=cvec(2 + j), scale=cvec(0 + j),
                        )
                        gB = gpool.tile([128, NCOL], f32, tag="gB")
                        nc.scalar.activation(
                            gB[:], ps1[:, :NCOL], Act.Relu,
                            bias=cvec(6 + j), scale=cvec(4 + j),
                        )
                        # y = (Q2*E1 + gA) - gB   (+D1tot folded downstream)
                        w1t = ypool.tile([128, NCOL], f32, tag="w1t")
                        nc.vector.scalar_tensor_tensor(
                            w1t[:], Q2p[j][:, :NCOL], cvec(8 + j), gA[:],
                            Alu.mult, Alu.add,
                        )
                        nc.vector.tensor_tensor(
                            y4[j][:, h, :], w1t[:], gB[:], Alu.subtract
                        )

                        yv = y4[j][:, h, :]
                        R2 = qpool.tile([128, NCOL], bf16, tag="r2")
                        nc.vector.tensor_scalar(
                            R2[:], yv, 7.5, cvec(10 + j), Alu.mult, Alu.add
                        )
                        # s2/2 in {-0.5,+0.5}: (y >= -D1tot) - 0.5
                        nc.vector.tensor_scalar(
                            s24[j][:, h, :], yv,
                            cvec(36 + j), 0.5, Alu.is_ge, Alu.subtract,
                        )
                        nc.vector.tensor_scalar(
                            rc24[j][:, h, :], R2[:],
                            207.0, 192.0, Alu.min, Alu.max,
                        )

                    # stage 2: 1x1 conv per group
                    for jj in range(4):
                        ps2 = pc2.tile([128, 512], f32, tag="ps2")
                        nc.tensor.matmul(
                            ps2[:, :NCOL],
                            W1S[:, jj * 128 : (jj + 1) * 128],
                            s24[0][:, h, :],
                            start=True,
                            stop=False,
                        )
                        nc.tensor.matmul(
                            ps2[:, :NCOL],
                            W1S[:, (4 + jj) * 128 : (5 + jj) * 128],
                            s24[1][:, h, :],
                            start=False,
                            stop=True,
                        )
                        nc.scalar.activation(
                            zA4[jj][:, h, :], ps2[:, :NCOL], Act.Relu,
                            bias=cvec(16 + jj), scale=cvec(12 + jj),
                        )
                        nc.scalar.activation(
                            zB4[jj][:, h, :], ps2[:, :NCOL], Act.Relu,
                            bias=cvec(24 + jj), scale=cvec(20 + jj),
                        )

                    for jj in range(4):
                        # u = rc2*E2 + D2tot (offset cancels in fp32
                        # internals, keeping u at O(1) for bf16)
                        U = zpool.tile([128, NCOL], bf16, tag="U")
                        nc.vector.tensor_scalar(
                            U[:], rc24[jj % 2][:, h, :],
                            cvec(28 + jj), cvec(32 + jj), Alu.mult, Alu.add,
                        )
                        T = zpool.tile([128, NCOL], bf16, tag="T")
                        nc.vector.tensor_tensor(
                            T[:], zA4[jj][:, h, :], zB4[jj][:, h, :],
                            Alu.subtract,
                        )
                        outS = opool.tile([128, NCOL], f32, tag="o")
                        nc.vector.tensor_tensor(outS[:], U[:], T[:], Alu.add)
                        nc.sync.dma_start(
                            out_d[2 * g : 2 * g + 2, jj].rearrange(
                                "s p x -> p s x"
                            ),
                            outS[:].rearrange("p (s x) -> p s x", s=2),
                        )

    nc.compile()
    _PROGRAM_CACHE["nc"] = nc
    return nc


def _prep_consts(
    w3, w1,
    bn1_m, bn1_v, bn1_w, bn1_b,
    bn2_m, bn2_v, bn2_w, bn2_b,
    sbn1_m, sbn1_v, sbn1_w, sbn1_b,
    sbn2_m, sbn2_v, sbn2_w, sbn2_b,
    rp1_gamma, rp1_beta, rp1_zeta,
    rp2_gamma, rp2_beta, rp2_zeta,
):
    f = np.float32
    eps = f(1e-5)
    w3 = w3.astype(f)
    w1 = w1.astype(f)

    inv1 = bn1_w / np.sqrt(bn1_v + eps)
    shift1 = bn1_b - bn1_m * inv1
    alpha3 = np.mean(np.abs(w3), axis=(1, 2, 3))
    s3 = np.where(w3 >= 0, f(1.0), f(-1.0))
    S3 = s3.sum(axis=(1, 2, 3))
    sinv1 = sbn1_w / np.sqrt(sbn1_v + eps)
    sshift1 = sbn1_b - sbn1_m * sinv1
    A1 = alpha3 * inv1
    base1 = shift1 - rp1_gamma
    sA1 = A1 * sinv1
    bA1 = base1 * sinv1
    q1 = rp1_beta * sinv1
    sB1 = -A1 * q1
    bB1 = -base1 * q1
    E1 = sinv1 / f(30.0)
    D1tot = rp1_zeta * sinv1 + sshift1 - sinv1 - f(768.0) * E1
    r2bias = f(199.5) + f(7.5) * D1tot

    inv2 = bn2_w / np.sqrt(bn2_v + eps)
    shift2 = bn2_b - bn2_m * inv2
    alpha1 = np.mean(np.abs(w1), axis=(1, 2, 3))
    s1 = np.where(w1 >= 0, f(1.0), f(-1.0))
    S1 = s1.sum(axis=(1, 2, 3))
    sinv2 = sbn2_w / np.sqrt(sbn2_v + eps)
    sshift2 = sbn2_b - sbn2_m * sinv2
    A2 = alpha1 * inv2
    base2 = shift2 - rp2_gamma
    sA2 = f(2.0) * A2 * sinv2
    bA2 = base2 * sinv2
    q2 = rp2_beta * sinv2
    sB2 = f(-2.0) * A2 * q2
    bB2 = -base2 * q2
    E2v = f(2.0 / 15.0) * sinv2
    D2tot = rp2_zeta * sinv2 + sshift2 - sinv2 - f(192.0) * E2v

    cv = np.zeros((128, 38), dtype=f)
    for j in range(2):
        sl = slice(j * 128, (j + 1) * 128)
        cv[:, 0 + j] = sA1[sl]
        cv[:, 2 + j] = bA1[sl]
        cv[:, 4 + j] = sB1[sl]
        cv[:, 6 + j] = bB1[sl]
        cv[:, 8 + j] = E1[sl]
        cv[:, 10 + j] = r2bias[sl]
        cv[:, 36 + j] = -D1tot[sl]
    for jj in range(4):
        sl = slice(jj * 128, (jj + 1) * 128)
        cv[:, 12 + jj] = sA2[sl]
        cv[:, 16 + jj] = bA2[sl]
        cv[:, 20 + jj] = sB2[sl]
        cv[:, 24 + jj] = bB2[sl]
        cv[:, 28 + jj] = E2v[sl]
        cv[:, 32 + jj] = D2tot[sl]

    # conv1 weights -> lhsT tiles [k, (j,c,kh,kw,m)] in bf16 sign form
    # o = j*128+m, i = c*128+k
    w3l = (
        s3.reshape(2, 128, 2, 128, 3, 3)
        .transpose(3, 0, 2, 4, 5, 1)  # [k, j, c, kh, kw, m]
        .reshape(128, 2 * 18 * 128)
        .astype(ml_dtypes.bfloat16)
    )
    # conv2 weights: [k, (c,jj,m)]; o = jj*128+m, i = c*128+k
    w1l = (
        s1.reshape(4, 128, 2, 128)
        .transpose(3, 2, 0, 1)  # [k, c, jj, m]
        .reshape(128, 2 * 4 * 128)
        .astype(ml_dtypes.bfloat16)
    )
    dg = np.eye(128, dtype=ml_dtypes.bfloat16)
    return w3l, w1l, cv, dg


def run(inputs, trace=False):
    from concourse import bass_utils

    nc = _build_program()
    x = np.asarray(inputs["x"], dtype=np.float32)
    w3l, w1l, cv, dg = _prep_consts(
        **{k: np.asarray(v, np.float32) for k, v in inputs.items() if k != "x"}
    )

    in_maps = []
    for core in range(N_CORES):
        xs = (
            x[core * B_PER_CORE : (core + 1) * B_PER_CORE]
            .reshape(B_PER_CORE, 2, 128, H * W)
            .copy()
        )
        in_maps.append({"xs": xs, "w3s": w3l, "w1s": w1l, "cv": cv, "dg": dg})

    res = bass_utils.run_bass_kernel_spmd(
        nc, in_maps, core_ids=list(range(N_CORES)), trace=trace
    )
    outs = [
        res.results[c]["out"].reshape(B_PER_CORE, COUT, HO, WO)
        for c in range(N_CORES)
    ]
    full = np.concatenate(outs, axis=0)
    return full, res


def kernel(**inputs):
    out, _ = run(inputs, trace=False)
    return out



# revision 14
# speedup vs baseline: 1.2725x; 1.2725x over previous
"""Trainium2 Bass kernel for the ReActNet-style binary conv building block.

Data-parallel across 8 NeuronCores (8 samples each). Key structure per
2-sample group:
  - scalar Sign act binarizes x -> fp8 +/-1 planes (zero-padded ring)
  - conv1 runs as 9 fp8 DoubleRow matmuls per half (K=256 per instr)
  - the 2x2 avgpool shortcut: bf16 quant grid (bf16(7.5x+199.5) rounds
    exactly), clipped, then summed by diag(E1) matmuls into PSUM
  - BN+RPReLU+shortcut-BN fold into a single per-channel Prelu
    activation (alpha = beta vector) per conv tile
  - stage 2: 1x1 conv as one fp8 DoubleRow matmul per 128-channel tile
  - final combine on DVE in bf16; output stored bf16, host casts to f32
"""

import sys

sys.path.insert(0, "/opt/trn_rl_repo")

import numpy as np
import ml_dtypes

B_PER_CORE = 8
N_CORES = 8
CIN = 256
COUT = 512
H = 28
W = 28
HO = 14
WO = 14
PIX = HO * WO  # 196
NG = 2  # samples per group
NGROUP = 4  # groups per core
NCOL = NG * PIX  # 392

# padded image layout rows 0..29, cols 0..31; interior at [1:29, 2:30]
PH, PW = 30, 32

_PROGRAM_CACHE = {}


def _build_program():
    if "nc" in _PROGRAM_CACHE:
        return _PROGRAM_CACHE["nc"]

    import concourse.bacc as bacc
    import concourse.tile as tile
    from concourse import mybir

    f32 = mybir.dt.float32
    bf16 = mybir.dt.bfloat16
    fp8 = mybir.dt.float8e4
    Alu = mybir.AluOpType
    Act = mybir.ActivationFunctionType
    DR = mybir.MatmulPerfMode.DoubleRow

    nc = bacc.Bacc(
        "TRN2",
        target_bir_lowering=False,
        debug=False,
        enable_asserts=False,
        num_devices=N_CORES,
    )

    xs_d = nc.dram_tensor("xs", [B_PER_CORE, 2, 128, H * W], f32, kind="ExternalInput")
    w3_d = nc.dram_tensor("w3f", [128, 9, 2, 2, 128], fp8, kind="ExternalInput")
    w1_d = nc.dram_tensor("w1f", [128, 2, 4, 128], fp8, kind="ExternalInput")
    dg_d = nc.dram_tensor("dg", [128, 128], bf16, kind="ExternalInput")
    cv_d = nc.dram_tensor("cv", [128, 32], f32, kind="ExternalInput")
    out_d = nc.dram_tensor(
        "out", [4, 128, B_PER_CORE, PIX], bf16, kind="ExternalOutput"
    )

    with tile.TileContext(nc) as tc:
        with (
            tc.tile_pool(name="consts", bufs=1) as cpool,
            tc.tile_pool(name="xin", bufs=2) as xpool,
            tc.tile_pool(name="rq", bufs=2) as rpool,
            tc.tile_pool(name="bpad", bufs=2) as bpool,
            tc.tile_pool(name="p1s", bufs=2) as p1pool,
            tc.tile_pool(name="ys", bufs=2) as ypool,
            tc.tile_pool(name="q2s", bufs=2) as qpool,
            tc.tile_pool(name="s2s", bufs=2) as spool,
            tc.tile_pool(name="p2s", bufs=2) as p2pool,
            tc.tile_pool(name="zs", bufs=2) as zpool,
            tc.tile_pool(name="pc1", bufs=2, space="PSUM") as pc1,
            tc.tile_pool(name="pq", bufs=2, space="PSUM") as pq,
            tc.tile_pool(name="pc2", bufs=2, space="PSUM") as pc2,
        ):
            W3F = cpool.tile([128, 9, 2, 2, 128], fp8)
            W1F = cpool.tile([128, 2, 4, 128], fp8)
            DG = cpool.tile([128, 128], bf16)
            CV = cpool.tile([128, 32], f32)
            nc.sync.dma_start(W3F[:], w3_d[:])
            nc.sync.dma_start(W1F[:], w1_d[:])
            nc.sync.dma_start(DG[:], dg_d[:])
            nc.sync.dma_start(CV[:], cv_d[:])

            def cvec(col):
                return CV[:, col : col + 1]

            # cv columns: 0,1 sA1 | 2,3 bA1 | 4,5 beta1 | 6,7 D1 | 8,9 r2bias
            # 10-13 sA2 | 14-17 bA2 | 18-21 beta2 | 22-25 E2 | 26-29 D2
            # 30,31 E1

            for g in range(NGROUP):
                X = xpool.tile([128, 2, NG, H * W], f32, tag="x")
                for si in range(NG):
                    nc.sync.dma_start(
                        X[:, :, si, :],
                        xs_d[NG * g + si].rearrange("c p hw -> p c hw"),
                    )
                Xv = X[:].rearrange("p c s (h w) -> p c s h w", h=H, w=W)

                BP = bpool.tile([128, 2, NG, PH, PW], fp8, tag="bpad")
                if g < 2:
                    # ring cells the conv taps read; interior rewritten
                    # each group, ring stays zero across buffer reuse
                    nc.gpsimd.memset(BP[:, :, :, 0, :], 0.0)
                    nc.gpsimd.memset(BP[:, :, :, 1:29, 1], 0.0)
                # binarize: sign(x) in {-1,+1} -> fp8, zero-padded ring
                nc.scalar.activation(
                    BP[:, :, :, 1:29, 2:30], Xv, Act.Sign
                )

                # quant grid: bf16(7.5x+199.5) rounds to ints; clip [192,207]
                R = rpool.tile([128, 2, NG, H * W], bf16, tag="r")
                nc.vector.tensor_scalar(
                    R[:], X[:], 7.5, 199.5, Alu.mult, Alu.add
                )
                RC = rpool.tile([128, 2, NG, H, W], bf16, tag="rc")
                nc.vector.tensor_scalar(
                    RC[:], R[:], 207.0, 192.0, Alu.min, Alu.max
                )

                # 2x2 sum-pool via identity matmuls into PSUM (exact ints)
                Q2p = [
                    pq.tile([128, 512], f32, tag=f"pq{j}", name=f"q2p_{g}_{j}")
                    for j in range(2)
                ]
                for j in range(2):
                    om = Q2p[j][:, :NCOL].rearrange(
                        "p (s y x) -> p s y x", s=NG, y=HO
                    )
                    for pp in range(4):
                        ph, pw = pp >> 1, pp & 1
                        nc.tensor.matmul(
                            om,
                            DG[:],
                            RC[:, j, :, ph::2, pw::2],
                            start=(pp == 0),
                            stop=(pp == 3),
                        )

                # conv1: 9 fp8 DoubleRow matmuls per half (K=256 each)
                y4 = ypool.tile([128, 2, NCOL], f32, tag="y4")
                S24 = spool.tile([128, 2, NCOL], fp8, tag="s24")
                for j in range(2):
                    ps1 = pc1.tile([128, 512], f32, tag="ps1")
                    for si in range(NG):
                        om = ps1[:, si * PIX : (si + 1) * PIX].rearrange(
                            "p (y x) -> p y x", y=HO
                        )
                        for t in range(9):
                            kh, kw = t // 3, t % 3
                            nc.tensor.matmul(
                                om,
                                W3F[:, t, :, j, :],
                                BP[:, :, si, kh : kh + 28, kw + 1 : kw + 29]
                                .rearrange(
                                    "p c (y a) (x b) -> p c y a x b", a=2, b=2
                                )[:, :, :, 0, :, 0],
                                start=(t == 0),
                                stop=(t == 8),
                                perf_mode=DR,
                            )
                    # fused BN+RPReLU+sBN1: P1 = prelu(sA1*t + bA1, beta1)
                    P1 = p1pool.tile([128, NCOL], f32, tag=f"p1{j}")
                    nc.scalar.activation(
                        P1[:], ps1[:, :NCOL], Act.Prelu,
                        bias=cvec(2 + j), scale=cvec(0 + j), alpha=cvec(4 + j),
                    )
                    # y = E1*pool + P1 with exact-f32 E1 (bf16 E1 in the
                    # pool diag costs ~1% end-to-end via quant boundaries)
                    nc.vector.scalar_tensor_tensor(
                        y4[:, j, :], Q2p[j][:, :NCOL], cvec(30 + j), P1[:],
                        Alu.mult, Alu.add,
                    )
                    # stage-2 binarize: sign(y + D1) -> fp8
                    nc.scalar.activation(
                        S24[:, j, :], y4[:, j, :], Act.Sign, bias=cvec(6 + j)
                    )

                # stage-2 shortcut quant: bf16 grid on y, clip
                R2 = qpool.tile([128, 2, NCOL], bf16, tag="r2")
                for j in range(2):
                    nc.vector.tensor_scalar(
                        R2[:, j, :], y4[:, j, :], 7.5, cvec(8 + j),
                        Alu.mult, Alu.add,
                    )
                RC2 = qpool.tile([128, 2, NCOL], bf16, tag="rc2")
                nc.vector.tensor_scalar(
                    RC2[:], R2[:], 207.0, 192.0, Alu.min, Alu.max
                )

                # stage 2: one fp8 DoubleRow matmul per output tile
                Z = zpool.tile([128, 4, NCOL], bf16, tag="z")
                for jj in range(4):
                    ps2 = pc2.tile([128, 512], f32, tag="ps2")
                    nc.tensor.matmul(
                        ps2[:, :NCOL],
                        W1F[:, :, jj, :],
                        S24[:],
                        start=True,
                        stop=True,
                        perf_mode=DR,
                    )
                    P2 = p2pool.tile([128, NCOL], bf16, tag="p2")
                    nc.scalar.activation(
                        P2[:], ps2[:, :NCOL], Act.Prelu,
                        bias=cvec(14 + jj), scale=cvec(10 + jj),
                        alpha=cvec(18 + jj),
                    )
                    U = p2pool.tile([128, NCOL], bf16, tag="u")
                    nc.vector.tensor_scalar(
                        U[:], RC2[:, jj % 2, :], cvec(22 + jj), cvec(26 + jj),
                        Alu.mult, Alu.add,
                    )
                    nc.vector.tensor_tensor(
                        Z[:, jj, :], U[:], P2[:], Alu.add
                    )

                nc.sync.dma_start(
                    out_d[:, :, NG * g : NG * g + NG, :].rearrange(
                        "jj p s x -> p jj (s x)"
                    ),
                    Z[:],
                )

    nc.compile()
    _PROGRAM_CACHE["nc"] = nc
    return nc


def _prep_consts(
    w3, w1,
    bn1_m, bn1_v, bn1_w, bn1_b,
    bn2_m, bn2_v, bn2_w, bn2_b,
    sbn1_m, sbn1_v, sbn1_w, sbn1_b,
    sbn2_m, sbn2_v, sbn2_w, sbn2_b,
    rp1_gamma, rp1_beta, rp1_zeta,
    rp2_gamma, rp2_beta, rp2_zeta,
):
    f = np.float32
    eps = f(1e-5)
    w3 = w3.astype(f)
    w1 = w1.astype(f)

    inv1 = bn1_w / np.sqrt(bn1_v + eps)
    shift1 = bn1_b - bn1_m * inv1
    alpha3 = np.mean(np.abs(w3), axis=(1, 2, 3))
    A1 = alpha3 * inv1
    base1 = shift1 - rp1_gamma
    sinv1 = sbn1_w / np.sqrt(sbn1_v + eps)
    sshift1 = sbn1_b - sbn1_m * sinv1
    sA1 = sinv1 * A1
    bA1 = sinv1 * base1
    E1 = sinv1 / f(30.0)
    D1 = sinv1 * rp1_zeta + sshift1 - f(798.0) * E1
    r2bias = f(199.5) + f(7.5) * D1

    inv2 = bn2_w / np.sqrt(bn2_v + eps)
    shift2 = bn2_b - bn2_m * inv2
    alpha1 = np.mean(np.abs(w1), axis=(1, 2, 3))
    A2 = alpha1 * inv2
    base2 = shift2 - rp2_gamma
    sinv2 = sbn2_w / np.sqrt(sbn2_v + eps)
    sshift2 = sbn2_b - sbn2_m * sinv2
    sA2 = sinv2 * A2
    bA2 = sinv2 * base2
    E2 = f(2.0 / 15.0) * sinv2
    D2 = sinv2 * rp2_zeta + sshift2 - f(199.5) * E2

    cv = np.zeros((128, 32), dtype=f)
    for j in range(2):
        sl = slice(j * 128, (j + 1) * 128)
        cv[:, 0 + j] = sA1[sl]
        cv[:, 2 + j] = bA1[sl]
        cv[:, 4 + j] = rp1_beta[sl]
        cv[:, 6 + j] = D1[sl]
        cv[:, 8 + j] = r2bias[sl]
        cv[:, 30 + j] = E1[sl]
    for jj in range(4):
        sl = slice(jj * 128, (jj + 1) * 128)
        cv[:, 10 + jj] = sA2[sl]
        cv[:, 14 + jj] = bA2[sl]
        cv[:, 18 + jj] = rp2_beta[sl]
        cv[:, 22 + jj] = E2[sl]
        cv[:, 26 + jj] = D2[sl]

    s3 = np.where(w3 >= 0, f(1.0), f(-1.0))
    # w3f[k, kh*3+kw, c, j, m] = s3[j*128+m, c*128+k, kh, kw]
    w3f = (
        s3.reshape(2, 128, 2, 128, 3, 3)
        .transpose(3, 4, 5, 2, 0, 1)
        .reshape(128, 9, 2, 2, 128)
        .astype(ml_dtypes.float8_e4m3)
    )
    s1 = np.where(w1 >= 0, f(1.0), f(-1.0))
    # w1f[k, c, jj, m] = s1[jj*128+m, c*128+k]
    w1f = (
        s1.reshape(4, 128, 2, 128)
        .transpose(3, 2, 0, 1)
        .astype(ml_dtypes.float8_e4m3)
    )
    dg = np.eye(128, dtype=ml_dtypes.bfloat16)
    return w3f, w1f, dg, cv


def run(inputs, trace=False):
    from concourse import bass_utils

    nc = _build_program()
    x = np.asarray(inputs["x"], dtype=np.float32)
    w3f, w1f, dg, cv = _prep_consts(
        **{k: np.asarray(v, np.float32) for k, v in inputs.items() if k != "x"}
    )

    in_maps = []
    for core in range(N_CORES):
        xs = (
            x[core * B_PER_CORE : (core + 1) * B_PER_CORE]
            .reshape(B_PER_CORE, 2, 128, H * W)
            .copy()
        )
        in_maps.append({"xs": xs, "w3f": w3f, "w1f": w1f, "dg": dg, "cv": cv})

    res = bass_utils.run_bass_kernel_spmd(
        nc, in_maps, core_ids=list(range(N_CORES)), trace=trace
    )
    outs = [
        res.results[c]["out"]
        .astype(np.float32)
        .transpose(2, 0, 1, 3)
        .reshape(B_PER_CORE, COUT, HO, WO)
        for c in range(N_CORES)
    ]
    full = np.concatenate(outs, axis=0)
    return full, res


def kernel(**inputs):
    out, _ = run(inputs, trace=False)
    return out
